# revision 1
# baseline (speedup 1.0000x reference)
"""Trainium2 Bass kernel for nn_AttentionSubsample (LeViT-style attention subsample).

Reference computation (per batch item):
  kv = hardswish(BN(Wkv @ x))               x: [256, 28, 28] -> kv: [384, 784]
  k  = per-head 16 ch of kv, v = 32 ch
  q  = hardswish(BN(Wq @ x[:, ::2, ::2]))   -> [128, 196] = 8 heads x 16
  attn = softmax(0.25 * q^T k + bias[h])    -> [196, 784] per (b, h)
  out  = v @ attn^T -> [256, 14, 14];  y = hardswish(BN(Wp @ hardswish(out)))

Sharding: data-parallel over batch B=64 -> 8 images per NeuronCore, weights
replicated, no collectives.

v3 design (fp16 fast path; fp32 matmuls cost 4 cyc/row on PE, fp16 cost 1):
  - x, conv weights, k/q/v outputs, E, ao, proj weights all fp16.
  - k_sb/qA/qB [128, *] strip layout: row 32g+j = head g dim j (j<16), head
    4+g dim j-16 (j>=16); zero-padded q halves give K=32 score matmuls at
    32-aligned tile_position (SBUF APs may only start at partition 0/32/64/96).
  - conv epilogues are 2 DVE ops: t = clamp(p,-3,3) [tensor_scalar], then
    out = (t+3)*p [scalar_tensor_tensor] = 6*hsw(p). The 6x folds into the
    exp scale for k/q (scale=SCALE/36, ebT pre-scaled 36x), into the vt
    constant column (36.0) for v, and into a host-side 1/6 for proj's y.
  - attention bias: identity-matmul accumulate of ebT (fp16) into score psum.
  - combine: vt [112, mc, h, 33] fp16, col 32 = 36.0 -> po[32] = 36*Z.
  - softmax divide: ACT copies po -> o_sb; DVE recip into f32r rz pairs
    [1, 392]; one N=392 fp32r broadcast matmul per 2 heads (fp32r is 1
    cyc/row at N>=256); u6/c1/ao on DVE; ao written into 2 stacked [128,196]
    tiles so proj is 2 K=128 matmuls + bias per 128-out chunk.
  - software pipelining: conv chunks of image b+1 are emitted interleaved
    into image b's attention head loop; psc double-buffered.
  - walrus allows ~1 sync wait per instruction: ldweights absorbers give PE
    the DMA/DVE ticks, an ACT absorber per head takes the PE tick before the
    exps, one ACT observer per image takes the DVE tick for the o_sb copies,
    and ring sizes are chosen so slot-reuse WARs collapse into already-
    observed engine clocks.
BN folded into conv weights host-side; conv bias added with K=1 matmuls.
repeat>1 builds the same pipeline over repeat*8 images for benchmarking only
(x tiles reused past 16 images, y DMA only for the first 8).
"""

import sys
import numpy as np

sys.path.insert(0, "/opt/trn_rl_repo")

import concourse.bass as bass
import concourse.tile as tile
from concourse.tile import add_dep_helper
from concourse import mybir
from concourse.bass_utils import run_bass_kernel_spmd
from concourse import library_config
from concourse.vector_clock import ScopedClock, VectorClock
from concourse.tile_sem_assignment import N_PROCS


def _split_drain_and_barrier(self, tick_clock, wait_clock):
    # Walrus in this environment allows only ~1 sync wait per instruction;
    # the stock kernel-tail drain carries one wait per live semaphore. Split
    # it into one single-wait drain per proc.
    g = tick_clock.global_clock
    for p in range(N_PROCS):
        if g[p] == 0:
            continue
        partial = VectorClock([g[q] if q == p else 0 for q in range(N_PROCS)])
        di = self.nc.sync.drain()
        wait_clock.add_sem_waits(di.ins, ScopedClock({None: partial}))
    self.nc.all_engine_barrier()
    popped = self.nc._tile_sem_poison_stack.pop()
    assert popped is self._sem_poison
    self.nc.clear_and_free_semaphores(list(self.sems.allocated().values()))
    self.nc.all_engine_barrier()


tile.TileContext._drain_and_barrier = _split_drain_and_barrier

F32 = mybir.dt.float32
F32R = mybir.dt.float32r
FP16 = mybir.dt.float16
ALU = mybir.AluOpType
ACTF = mybir.ActivationFunctionType

EPS = 1e-5
SCALE = 16 ** -0.5  # = 0.25
H, KD, D, RES, RES_, STRIDE = 8, 16, 32, 28, 14, 2
N_TOK, N_TOK_ = RES * RES, RES_ * RES_  # 784, 196
N_CORES = 8
B_PER_CORE = 8
MC = 7  # m-chunks of 112 over 784
MCS = 112

_CACHE = {}


def _strip_perm():
    """Channel permutation: strip row 32g+j = head g (j<16) else head 4+g."""
    perm = np.zeros(128, dtype=np.int64)
    for g in range(4):
        for j in range(16):
            perm[32 * g + j] = g * 16 + j
            perm[32 * g + 16 + j] = (4 + g) * 16 + j
    return perm  # new_row -> old (head-major) row


def _prepare_host(inputs):
    x = np.asarray(inputs["x"], dtype=np.float32)          # [64, 256, 28, 28]
    B = x.shape[0]

    def fold(w, g, b, m, v):
        s = g / np.sqrt(v + EPS)
        return (s[:, None] * w).astype(np.float32), (b - m * s).astype(np.float32)

    wkv, bkv = fold(inputs["kv_w"], inputs["kv_g"], inputs["kv_b"],
                    inputs["kv_m"], inputs["kv_v"])        # [384, 256]
    wq, bq = fold(inputs["q_w"], inputs["q_g"], inputs["q_b"],
                  inputs["q_m"], inputs["q_v"])            # [128, 256]
    wp, bp = fold(inputs["proj_w"], inputs["proj_g"], inputs["proj_b"],
                  inputs["proj_m"], inputs["proj_v"])      # [512, 256]

    kv3 = wkv.reshape(H, KD + D, 256)
    bkv3 = bkv.reshape(H, KD + D)
    wk = kv3[:, :KD, :].reshape(H * KD, 256)               # head-major k weights
    bk = bkv3[:, :KD].reshape(H * KD)
    wv = kv3[:, KD:, :].reshape(H * D, 256)                # v channels h*32+d
    bv = bkv3[:, KD:].reshape(H * D)

    perm = _strip_perm()
    wk_s, bk_s = wk[perm], bk[perm]                        # strip-shuffled
    wq_s, bq_s = wq[perm], bq[perm]

    # Zero-padded halves: qA keeps lower 16 of each 32-strip, qB the upper.
    # Zeroing weights AND bias makes conv output exactly 0 there; hsw(0)=0.
    half_lo = np.zeros(128, dtype=np.float32)
    for g in range(4):
        half_lo[32 * g: 32 * g + 16] = 1.0
    wqA = wq_s * half_lo[:, None]
    wqB = wq_s * (1.0 - half_lo)[:, None]
    bqA = bq_s * half_lo
    bqB = bq_s * (1.0 - half_lo)

    # lhsT layouts: [K-chunk, 128, M], fp16
    wkT = np.ascontiguousarray(wk_s.T.reshape(2, 128, 128).astype(np.float16))
    wqTA = np.ascontiguousarray(wqA.T.reshape(2, 128, 128).astype(np.float16))
    wqTB = np.ascontiguousarray(wqB.T.reshape(2, 128, 128).astype(np.float16))
    wvT = np.ascontiguousarray(wv.T.reshape(2, 128, 256).astype(np.float16))
    # proj: stacked quads. ao_stack0 rows 32g+d = head g dim d (= input chans
    # 0..127 in original order); stack1 = heads 4..7 (chans 128..255).
    wpT = np.ascontiguousarray(
        (6.0 * wp).T.reshape(2, 128, 512).astype(np.float16))

    # bias transposed, chunk-major cols: ebT[p, h, mc*196 + n] =
    # (bias/SCALE)[h, n, mc*112 + p]  (fp16)
    idxs = np.asarray(inputs["bias_idxs"])                 # [196, 784] int32
    ab = np.asarray(inputs["ab"], dtype=np.float32)        # [8, n_off]
    bias = ab[:, idxs]                                     # [8, 196, 784]
    bT = (36.0 * bias / SCALE).transpose(0, 2, 1).reshape(H, MC, MCS, N_TOK_)
    ebT = np.ascontiguousarray(
        bT.transpose(2, 0, 1, 3).reshape(MCS, H, MC * N_TOK_).astype(np.float16))
    ident = np.ascontiguousarray(np.eye(MCS, dtype=np.float16))

    xf = x.reshape(B, 256, N_TOK)

    shared = dict(
        wkT=wkT, wqTA=wqTA, wqTB=wqTB, wvT=wvT, wpT=wpT,
        ones32=np.ones((1, 32), dtype=np.float32),
        bk=bk_s.reshape(1, 128).astype(np.float16),
        bqA=bqA.reshape(1, 128).astype(np.float16),
        bqB=bqB.reshape(1, 128).astype(np.float16),
        bv=bv.reshape(1, 256).astype(np.float16),
        bp=bp.reshape(1, 512).astype(np.float16),
        ebT=ebT, ident=ident,
    )
    in_maps = []
    for c in range(N_CORES):
        sl = slice(c * B_PER_CORE, (c + 1) * B_PER_CORE)
        m = dict(shared)
        m["x"] = np.ascontiguousarray(
            xf[sl].reshape(B_PER_CORE, 2, 128, N_TOK).astype(np.float16))
        in_maps.append(m)
    return in_maps


def build_bass(repeat=1):
    nc = bass.Bass()

    x_d = nc.dram_tensor("x", [B_PER_CORE, 2, 128, N_TOK], FP16, kind="ExternalInput")
    wkT_d = nc.dram_tensor("wkT", [2, 128, 128], FP16, kind="ExternalInput")
    wqTA_d = nc.dram_tensor("wqTA", [2, 128, 128], FP16, kind="ExternalInput")
    wqTB_d = nc.dram_tensor("wqTB", [2, 128, 128], FP16, kind="ExternalInput")
    wvT_d = nc.dram_tensor("wvT", [2, 128, 256], FP16, kind="ExternalInput")
    wpT_d = nc.dram_tensor("wpT", [2, 128, 512], FP16, kind="ExternalInput")
    bk_d = nc.dram_tensor("bk", [1, 128], FP16, kind="ExternalInput")
    bqA_d = nc.dram_tensor("bqA", [1, 128], FP16, kind="ExternalInput")
    bqB_d = nc.dram_tensor("bqB", [1, 128], FP16, kind="ExternalInput")
    bv_d = nc.dram_tensor("bv", [1, 256], FP16, kind="ExternalInput")
    bp_d = nc.dram_tensor("bp", [1, 512], FP16, kind="ExternalInput")
    ebT_d = nc.dram_tensor("ebT", [MCS, H, MC * N_TOK_], FP16, kind="ExternalInput")
    ones32_d = nc.dram_tensor("ones32", [1, 32], F32R, kind="ExternalInput")
    ident_d = nc.dram_tensor("ident", [MCS, MCS], FP16, kind="ExternalInput")
    n_img = repeat * B_PER_CORE
    y_d = nc.dram_tensor("y", [n_img, 128, 4, N_TOK_], F32, kind="ExternalOutput")

    with tile.TileContext(nc) as tc:
        with (
            tc.tile_pool(name="consts", bufs=1) as consts,
            tc.tile_pool(name="xp", bufs=8) as xp,
            tc.tile_pool(name="kp", bufs=3) as kp,
            tc.tile_pool(name="qp", bufs=3) as qp,
            tc.tile_pool(name="vp", bufs=3) as vp,
            tc.tile_pool(name="tp", bufs=6) as tp,
            tc.tile_pool(name="ep", bufs=16) as ep,
            tc.tile_pool(name="att", bufs=2) as attp,
            tc.tile_pool(name="yp", bufs=8) as yp,
            tc.tile_pool(name="psc", bufs=2, space="PSUM") as psc,
            tc.tile_pool(name="pss", bufs=4, space="PSUM") as pss,
            tc.tile_pool(name="psa", bufs=1, space="PSUM") as psa,
            tc.tile_pool(name="psz", bufs=1, space="PSUM") as psz,
        ):
            # ---- load weights / constants (resident) ----
            wk = [consts.tile([128, 128], FP16, tag=f"wk{i}", name=f"wk{i}") for i in range(2)]
            wqa = [consts.tile([128, 128], FP16, tag=f"wqa{i}", name=f"wqa{i}") for i in range(2)]
            wqb = [consts.tile([128, 128], FP16, tag=f"wqb{i}", name=f"wqb{i}") for i in range(2)]
            wv = [consts.tile([128, 256], FP16, tag=f"wv{i}", name=f"wv{i}") for i in range(2)]
            wp = [consts.tile([128, 512], FP16, tag=f"wp{i}", name=f"wp{i}") for i in range(2)]
            for i in range(2):
                nc.sync.dma_start(out=wk[i], in_=wkT_d[i])
                nc.sync.dma_start(out=wqa[i], in_=wqTA_d[i])
                nc.sync.dma_start(out=wqb[i], in_=wqTB_d[i])
                nc.sync.dma_start(out=wv[i], in_=wvT_d[i])
                nc.sync.dma_start(out=wp[i], in_=wpT_d[i])
            bk = consts.tile([1, 128], FP16, tag="bk", name="bk")
            bqa = consts.tile([1, 128], FP16, tag="bqa", name="bqa")
            bqb = consts.tile([1, 128], FP16, tag="bqb", name="bqb")
            bv = consts.tile([1, 256], FP16, tag="bv", name="bv")
            bp = consts.tile([1, 512], FP16, tag="bp", name="bp")
            nc.sync.dma_start(out=bk, in_=bk_d[:])
            nc.sync.dma_start(out=bqa, in_=bqA_d[:])
            nc.sync.dma_start(out=bqb, in_=bqB_d[:])
            nc.sync.dma_start(out=bv, in_=bv_d[:])
            nc.sync.dma_start(out=bp, in_=bp_d[:])
            ebT = consts.tile([MCS, H, MC * N_TOK_], FP16, tag="ebT", name="ebT")
            for qi in range(4):
                nc.sync.dma_start(out=ebT[:, 2 * qi: 2 * qi + 2, :],
                                  in_=ebT_d[:, 2 * qi: 2 * qi + 2, :])
            ident = consts.tile([MCS, MCS], FP16, tag="ident", name="ident")
            nc.sync.dma_start(out=ident, in_=ident_d[:])

            ones_f = consts.tile([1, N_TOK], FP16, tag="ones_f", name="ones_f")
            nc.vector.memset(ones_f, 1.0)
            ones32 = consts.tile([1, 32], F32R, tag="ones32", name="ones32")
            nc.sync.dma_start(out=ones32, in_=ones32_d[:])
            # per-head ACT-side PE-tick absorber outputs (never reused)
            scr_act = consts.tile([1, n_img * (H + 1)], F32, tag="scr_act",
                                  name="scr_act")
            # cross-proc observation targets: DVE absorbs Pool ticks, Pool
            # absorbs PE ticks (writes never read back)
            scr_dve = consts.tile([1, 2], FP16, tag="scr_dve", name="scr_dve")
            scr_pool = consts.tile([1, 2], FP16, tag="scr_pool", name="scr_pool")

            # Wait-absorbers: walrus allows at most ONE sync wait per
            # instruction. A standalone ldweights reading one element of a
            # tile ticks the PE clock after that tile's producer; later PE
            # instructions then inherit the dep through the engine clock.
            # (fp16 tiles only -- ldweights refuses fp32.)
            def absorb(t, tile_position=None):
                sl = t[tuple(slice(0, 1) for _ in t.shape)]
                return nc.tensor.ldweights(weights=sl, tile_position=tile_position)

            for t in (wk[0], wk[1], wqa[0], wqa[1], wqb[0], wqb[1],
                      wv[0], wv[1], wp[0], wp[1],
                      bk, bqa, bqb, bv, bp, ident, ones_f):
                absorb(t)

            absorb(ebT)

            # ---------- per-image emission helpers ----------
            xt_cache = {}

            def make_conv_units(b, b_seq=0):
                """DMA x and build conv-chunk closures for image b. Returns
                (tiles dict, unit list); units are emitted interleaved into
                the previous image's attention phase. Beyond 16 images (bench
                repeat mode) x tiles are reused without reloading."""
                if b_seq < 16:
                    xt = [xp.tile([128, N_TOK], FP16, tag=f"x{i}", name=f"x{i}",
                                  bufs=min(8 * repeat, 16))
                          for i in range(2)]
                    xt_cache[b_seq % 16] = xt
                    for i in range(2):
                        nc.sync.dma_start(out=xt[i], in_=x_d[b, i])
                        absorb(xt[i])
                else:
                    xt = xt_cache[b_seq % 16]
                xst = [bass.AP(tensor=xt[i].tensor, offset=xt[i].offset,
                               ap=[xt[i].ap[0], [2 * RES, RES_], [2, RES_]])
                       for i in range(2)]
                k_sb = kp.tile([128, N_TOK], FP16, tag="k", name="k")
                qA = qp.tile([128, N_TOK_], FP16, tag="qA", name="qA")
                qB = qp.tile([128, N_TOK_], FP16, tag="qB", name="qB")
                vt = vp.tile([MCS, MC, H, 33], FP16, tag="vt", name="vt")

                units = []

                def k_half(nh):
                    def emit():
                        pk = psc.tile([128, 392], F32, tag="conv", name="conv")
                        sl = slice(nh * 392, (nh + 1) * 392)
                        nc.tensor.matmul(pk, wk[0], xt[0][:, sl], start=True, stop=False)
                        nc.tensor.matmul(pk, wk[1], xt[1][:, sl], start=False, stop=False)
                        nc.tensor.matmul(pk, bk, ones_f[:, :392], start=False, stop=True)
                        tk = tp.tile([128, 392], FP16, tag="tk", name="tk")
                        nc.vector.tensor_scalar(tk, pk, -3.0, 3.0, ALU.max, ALU.min)
                        nc.vector.scalar_tensor_tensor(
                            k_sb[:, sl], tk, 3.0, pk, ALU.add, ALU.mult)
                    return emit

                def q_conv(wset, bset, qdst):
                    def emit():
                        pq = psc.tile([128, 392], F32, tag="conv", name="conv")
                        pq196 = pq[:, :N_TOK_]
                        nc.tensor.matmul(pq196, wset[0], xst[0], start=True, stop=False)
                        nc.tensor.matmul(pq196, wset[1], xst[1], start=False, stop=False)
                        nc.tensor.matmul(pq196, bset, ones_f[:, :N_TOK_],
                                         start=False, stop=True)
                        tq = tp.tile([128, 392], FP16, tag="tk", name="tq")
                        tq196 = tq[:, :N_TOK_]
                        nc.vector.tensor_scalar(tq196, pq196, -3.0, 3.0,
                                                ALU.max, ALU.min)
                        nc.vector.scalar_tensor_tensor(
                            qdst, tq196, 3.0, pq196, ALU.add, ALU.mult)
                    return emit

                def v_head_col():
                    def emit():
                        nc.vector.memset(vt[:, :, :, 32:33], 36.0)
                    return emit

                def v_chunk(mc):
                    def emit():
                        pv = psc.tile([128, 392], F32, tag="conv", name="conv")
                        pv = pv[:MCS, :256]
                        msl = slice(mc * MCS, (mc + 1) * MCS)
                        nc.tensor.matmul(pv, xt[0][:, msl], wv[0], start=True, stop=False)
                        nc.tensor.matmul(pv, xt[1][:, msl], wv[1], start=False, stop=False)
                        nc.tensor.matmul(pv, ones_f[:, :MCS], bv, start=False, stop=True)
                        tv = tp.tile([MCS, 256], FP16, tag="tv", name="tv")
                        nc.vector.tensor_scalar(tv, pv, -3.0, 3.0, ALU.max, ALU.min)
                        nc.vector.scalar_tensor_tensor(
                            vt[:, mc, :, 0:32],
                            tv.rearrange("p (h d) -> p h d", h=H), 3.0,
                            pv.rearrange("p (h d) -> p h d", h=H),
                            ALU.add, ALU.mult)
                    return emit

                units.append(v_head_col())
                units.append(k_half(0))
                units.append(k_half(1))
                units.append(q_conv(wqa, bqa, qA))
                units.append(q_conv(wqb, bqb, qB))
                for mc in range(MC):
                    units.append(v_chunk(mc))
                return dict(k=k_sb, qA=qA, qB=qB, vt=vt), units

            state = dict(prev_et=None, prev_oc=None)

            def emit_attention(b, tiles, units):
                """Attention + proj for image b; `units` (next image's conv
                chunks) are interleaved into the per-head stalls."""
                k_sb, qA, qB, vt = tiles["k"], tiles["qA"], tiles["qB"], tiles["vt"]
                ui = iter(units)

                def emit_next_unit(n=1):
                    for _ in range(n):
                        u = next(ui, None)
                        if u is not None:
                            u()

                # image gates: PE observes the conv-epilogue DVE ticks once
                for gt in (k_sb, qA, qB):
                    absorb(gt)
                if state.get("prev_u6") is not None:
                    # ACT observes the latest DVE division tick once per image
                    # so the o_sb copy's slot WAR (vs DVE readers of the
                    # previous image) elides to a single PE wait
                    nc.scalar.copy(scr_act[0:1, n_img * H + b: n_img * H + b + 1],
                                   state["prev_u6"][0:1, 0:1])
                ao_stack = [
                    attp.tile([128, N_TOK_], FP16, tag="ao0", name="ao0"),
                    attp.tile([128, N_TOK_], FP16, tag="ao1", name="ao1"),
                ]
                pend = None
                rz = None
                ao_ops = []
                for h in range(H):
                    g, hf = h % 4, h // 4
                    qt = qA if h < 4 else qB
                    rows = slice(32 * g, 32 * g + 32)
                    gate = None
                    if state["prev_et"] is not None:
                        # PE observes the previous head's last exp tick,
                        # eliding ACT-WAR waits on pss slot reuse
                        gate = absorb(state["prev_et"])
                    pstiles = []
                    for t in range(4):
                        ps = pss.tile([MCS, 2 * N_TOK_], F32, tag="score",
                                      name="score")
                        pstiles.append(ps)
                        fd = 2 * N_TOK_ if t < 3 else N_TOK_
                        for half in range(2):
                            mc = 2 * t + half
                            if mc >= MC:
                                break
                            msl = slice(mc * MCS, (mc + 1) * MCS)
                            nsl = slice(half * N_TOK_, (half + 1) * N_TOK_)
                            mm1 = nc.tensor.matmul(ps[:, nsl], k_sb[rows, msl],
                                                   qt[rows, :],
                                                   start=(half == 0), stop=False,
                                                   tile_position=(32 * g, 0),
                                                   skip_group_check=True)
                            if gate is not None and half == 0 and t == 0:
                                add_dep_helper(mm1.ins, gate.ins, True,
                                               "gate before slot reuse")
                        esl = slice(t * 2 * N_TOK_, t * 2 * N_TOK_ + fd)
                        nc.tensor.matmul(ps[:, :fd], ident, ebT[:, h, esl],
                                         start=False, stop=True,
                                         skip_group_check=True)
                    # one ACT absorber: after this, exps need no PE wait
                    nc.scalar.copy(scr_act[0:1, b * H + h: b * H + h + 1],
                                   pstiles[3][0:1, 0:1])
                    etiles = []
                    for t in range(4):
                        fd = 2 * N_TOK_ if t < 3 else N_TOK_
                        Et = ep.tile([MCS, 2 * N_TOK_], FP16, tag="E", name="E")
                        nc.scalar.activation(Et[:, :fd], pstiles[t][:, :fd],
                                             ACTF.Exp, bias=0.0, scale=SCALE / 36.0)
                        etiles.append(Et)
                    # fill the PE stall during this head's exps with one
                    # conv chunk of the next image
                    emit_next_unit()
                    state["prev_et"] = etiles[3]
                    po = psa.tile([33, N_TOK_], F32, tag="att", name="att")
                    ogate = None
                    if state["prev_oc"] is not None:
                        # observe the o_sb copy (DVE) that released this slot
                        ogate = absorb(ones_f)
                        add_dep_helper(ogate.ins, state["prev_oc"].ins, True,
                                       "osb copy gate")
                    # PE dummy takes the slot WAW; real matmuls then only
                    # wait on their E (ACT) tick
                    podum = nc.tensor.matmul(po[0:1, 0:1], ones_f[0:1, 0:1],
                                             ones_f[0:1, 0:1], start=True,
                                             stop=True, skip_group_check=True)
                    if ogate is not None:
                        add_dep_helper(podum.ins, ogate.ins, True,
                                       "osb gate before po dummy")
                    for mc in range(MC):
                        nsl = slice((mc % 2) * N_TOK_, (mc % 2 + 1) * N_TOK_)
                        nc.tensor.matmul(po, vt[:, mc, h, :],
                                         etiles[mc // 2][:, nsl],
                                         start=(mc == 0), stop=(mc == MC - 1),
                                         skip_group_check=True)
                    o_sb = attp.tile([33, N_TOK_], F32, tag="o", name="o", bufs=8)
                    oc = nc.scalar.copy(o_sb, po)
                    state["prev_oc"] = oc
                    if h % 2 == 0:
                        rz = attp.tile([1, 2 * N_TOK_], F32R, tag="rz", name="rz", bufs=4)
                    rsl = slice((h % 2) * N_TOK_, (h % 2 + 1) * N_TOK_)
                    with nc.allow_low_precision(reason="f32r rz for 1-cyc bcast"):
                        rc = nc.vector.reciprocal(out=rz[:, rsl],
                                                  in_=o_sb[32:33, :])
                    if h % 2 == 0:
                        pend = (h, o_sb)
                        continue
                    # odd head: one N=392 fp32r broadcast matmul for the pair,
                    # then both heads' division + hardswish-gate epilogues
                    pzb = psz.tile([32, 2 * N_TOK_], F32, tag="zb", name="zb")
                    nc.tensor.matmul(pzb, ones32, rz, start=True, stop=True,
                                     skip_group_check=True)
                    for pi, (hh, osb_hh) in enumerate([pend, (h, o_sb)]):
                        gg, hhf = hh % 4, hh // 4
                        drows = slice(32 * gg, 32 * gg + 32)
                        zsl = slice(pi * N_TOK_, (pi + 1) * N_TOK_)
                        u6 = attp.tile([32, N_TOK_], FP16, tag="u6", name="u6", bufs=8)
                        nc.vector.tensor_mul(u6, osb_hh[0:32, :], pzb[:, zsl])
                        c1 = attp.tile([32, N_TOK_], FP16, tag="c1", name="c1", bufs=8)
                        nc.vector.tensor_scalar(c1, u6, 0.5, 0.0, ALU.add, ALU.max)
                        aow = nc.vector.scalar_tensor_tensor(
                            ao_stack[hhf][drows, :], c1, 1.0, u6,
                            ALU.min, ALU.mult)
                        ao_ops.append(aow)
                        state["prev_u6"] = u6
                    pend = None

                # ---- proj conv: 2 stacked K=128 matmuls per 128-out chunk ----
                prj_abs = []
                for st in range(2):
                    # observe the LAST Pool writer of each stack (head 3 / 7
                    # write rows 96:128); Pool FIFO makes that tick cover all
                    # four quadrant writers
                    pa = absorb(ao_stack[st][96:97, :], tile_position=(96, 0))
                    prj_abs.append(pa)
                y_sb = yp.tile([128, 4, N_TOK_], F32,
                               tag="y" if bb < B_PER_CORE else "y2", name="y",
                               bufs=8)
                for mt in range(4):
                    pp = psc.tile([128, 392], F32, tag="conv", name="conv")
                    pp196 = pp[:, :N_TOK_]
                    csl = slice(mt * 128, (mt + 1) * 128)
                    for st in range(2):
                        pm = nc.tensor.matmul(pp196, wp[st][:, csl],
                                              ao_stack[st],
                                              start=(st == 0), stop=False,
                                              skip_group_check=True)
                        if mt == 0 and st == 0:
                            for pa in prj_abs:
                                add_dep_helper(pm.ins, pa.ins, False,
                                               "proj after ao gates")
                    nc.tensor.matmul(pp196, bp[:, csl], ones_f[:, :N_TOK_],
                                     start=False, stop=True,
                                     skip_group_check=True)
                    tpj = tp.tile([128, 392], FP16, tag="tk", name="tpj")
                    tpj196 = tpj[:, :N_TOK_]
                    nc.vector.tensor_scalar(tpj196, pp196, -3.0, 3.0,
                                            ALU.max, ALU.min)
                    nc.vector.scalar_tensor_tensor(
                        y_sb[:, mt, :], tpj196, 3.0, pp196, ALU.add, ALU.mult)
                    emit_next_unit()
                if bb < B_PER_CORE:
                    nc.gpsimd.dma_start(out=y_d[bb], in_=y_sb)
                # drain any units the head loop didn't consume
                emit_next_unit(len(units))

            # ---------- main pipeline ----------
            tiles, units = make_conv_units(0, 0)
            for u in units:
                u()
            for bb in range(n_img):
                if bb + 1 < n_img:
                    ntiles, nunits = make_conv_units((bb + 1) % B_PER_CORE, bb + 1)
                else:
                    ntiles, nunits = None, []
                emit_attention(bb, tiles, nunits)
                tiles = ntiles

    return nc


def _postprocess(y):
    # y dram: [8, 128, 4, 196] holding 6*hsw -> [8, 512, 14, 14]
    return y.transpose(0, 2, 1, 3).reshape(B_PER_CORE, 512, RES_, RES_) * (1.0 / 6.0)


def _run(inputs, trace=False):
    in_maps = _prepare_host(inputs)
    if "nc" not in _CACHE:
        _CACHE["nc"] = build_bass()
    nc = _CACHE["nc"]
    res = run_bass_kernel_spmd(nc, in_maps, list(range(N_CORES)), trace=trace)
    outs = []
    for c in range(N_CORES):
        outs.append(_postprocess(np.asarray(res.results[c]["y"])))
    full = np.concatenate(outs, axis=0).astype(np.float32)
    return full, res


def kernel(**inputs) -> np.ndarray:
    full, _ = _run(inputs, trace=False)
    return full



# revision 13
# speedup vs baseline: 1.6938x; 1.6938x over previous
"""Trainium2 Bass kernel for nn_AttentionSubsample (LeViT-style attention subsample).

Reference computation (per batch item):
  kv = hardswish(BN(Wkv @ x))               x: [256, 28, 28] -> kv: [384, 784]
  k  = per-head 16 ch of kv, v = 32 ch
  q  = hardswish(BN(Wq @ x[:, ::2, ::2]))   -> [128, 196] = 8 heads x 16
  attn = softmax(0.25 * q^T k + bias[h])    -> [196, 784] per (b, h)
  out  = v @ attn^T -> [256, 14, 14];  y = hardswish(BN(Wp @ hardswish(out)))

Sharding: data-parallel over batch B=64 -> 8 images per NeuronCore, weights
replicated, no collectives.

v4 design (K=128 everywhere: the PE clock-gate throttles to 1.2 GHz when the
contraction depth is small; K=128 matmuls sustain 2.4 GHz at any free dim):
  - scores per head-PAIR: rhs qm[g] [128, 392] holds q of head g in strip-g's
    lower 16 rows at cols 0:196 and q of head g+4 in the upper 16 rows at
    cols 196:392, all other rows ZERO.  One K=128 matmul against the full
    k_sb chunk [128, 112] computes both heads' scores (zero rows mask the
    other heads).  Strip row 32g+j = head g dim j (j<16), head 4+g dim j-16.
  - attention bias accumulated by a K=128 identity matmul from ebT
    [128(pad), 4, 7, 392] (rows 112:127 zero).
  - conv biases via replicated-bias weights (b/128 in all 128 rows) against
    an all-ones rhs: K=128 instead of K=1 (K=1 would throttle the clock).
  - combine per pair into po [97, 196]: g-block cols 0:33 of vt -> rows 0:33
    (Z=36*sum(exp) at row 32), g4-block cols 34:67 -> rows 64:97 (Z at 96).
    vt [128, 7, 4, 68] with rows 112:127 zeroed; E tiles [128, 392] with
    rows 112:127 zeroed once per slot, so both matmul operands are K=128.
  - 1/Z: per pair two DVE row-copies gather Z rows (o_sb[32], o_sb[96]) into
    zall [1, 1568]; one ACT Ln + one ACT Exp(-x) -> rz16 [1, 1568] fp16 =
    1/(36Z) for all 8 heads; per pair one all-fp16 K=1... K=32-row ones
    broadcast matmul pzb [32, 392]; u6 = o * pzb on DVE.
  - conv epilogues are 2 DVE ops: t = clamp(p,-3,3), out = (t+3)*p = 6hsw(p);
    the 6x factors fold exactly as in v3 (exp scale SCALE/36, vt Z-col 36.0,
    wpT pre-scaled 6x, host-side 1/6).
  - walrus allows ~1 sync wait per instruction: ldweights absorbers give PE
    the DMA/DVE ticks; an ACT scr_act copy per pair absorbs the DVE tick
    before the o_sb copy; ldweights absorbers before bcast matmuls take the
    u6 DVE ticks.
BN folded into conv weights host-side. Softmax denominator rides the combine
matmul as a 36.0 column of vt.
"""

import sys
import numpy as np

sys.path.insert(0, "/opt/trn_rl_repo")

import concourse.bass as bass
import concourse.tile as tile
from concourse.tile import add_dep_helper
from concourse import mybir
from concourse.bass_utils import run_bass_kernel_spmd
from concourse.vector_clock import ScopedClock, VectorClock
from concourse.tile_sem_assignment import N_PROCS


def _split_drain_and_barrier(self, tick_clock, wait_clock):
    # Walrus in this environment allows only ~1 sync wait per instruction;
    # the stock kernel-tail drain carries one wait per live semaphore. Split
    # it into one single-wait drain per proc.
    g = tick_clock.global_clock
    for p in range(N_PROCS):
        if g[p] == 0:
            continue
        partial = VectorClock([g[q] if q == p else 0 for q in range(N_PROCS)])
        di = self.nc.sync.drain()
        wait_clock.add_sem_waits(di.ins, ScopedClock({None: partial}))
    self.nc.all_engine_barrier()
    popped = self.nc._tile_sem_poison_stack.pop()
    assert popped is self._sem_poison
    self.nc.clear_and_free_semaphores(list(self.sems.allocated().values()))
    self.nc.all_engine_barrier()


tile.TileContext._drain_and_barrier = _split_drain_and_barrier

F32 = mybir.dt.float32
FP16 = mybir.dt.float16
ALU = mybir.AluOpType
ACTF = mybir.ActivationFunctionType

EPS = 1e-5
SCALE = 16 ** -0.5  # = 0.25
H, KD, D, RES, RES_, STRIDE = 8, 16, 32, 28, 14, 2
N_TOK, N_TOK_ = RES * RES, RES_ * RES_  # 784, 196
N_CORES = 8
B_PER_CORE = 8
MC = 7  # m-chunks of 112 over 784
MCS = 112

_CACHE = {}


def _strip_perm():
    """Channel permutation: strip row 32g+j = head g (j<16) else head 4+g."""
    perm = np.zeros(128, dtype=np.int64)
    for g in range(4):
        for j in range(16):
            perm[32 * g + j] = g * 16 + j
            perm[32 * g + 16 + j] = (4 + g) * 16 + j
    return perm  # new_row -> old (head-major) row


def _prepare_host(inputs):
    x = np.asarray(inputs["x"], dtype=np.float32)          # [64, 256, 28, 28]
    B = x.shape[0]

    def fold(w, g, b, m, v):
        s = g / np.sqrt(v + EPS)
        return (s[:, None] * w).astype(np.float32), (b - m * s).astype(np.float32)

    wkv, bkv = fold(inputs["kv_w"], inputs["kv_g"], inputs["kv_b"],
                    inputs["kv_m"], inputs["kv_v"])        # [384, 256]
    wq, bq = fold(inputs["q_w"], inputs["q_g"], inputs["q_b"],
                  inputs["q_m"], inputs["q_v"])            # [128, 256]
    wp, bp = fold(inputs["proj_w"], inputs["proj_g"], inputs["proj_b"],
                  inputs["proj_m"], inputs["proj_v"])      # [512, 256]

    kv3 = wkv.reshape(H, KD + D, 256)
    bkv3 = bkv.reshape(H, KD + D)
    wk = kv3[:, :KD, :].reshape(H * KD, 256)               # head-major k weights
    bk = bkv3[:, :KD].reshape(H * KD)
    wv = kv3[:, KD:, :].reshape(H * D, 256)                # v channels h*32+d
    bv = bkv3[:, KD:].reshape(H * D)

    perm = _strip_perm()
    wk_s, bk_s = wk[perm], bk[perm]                        # strip-shuffled
    wq_s, bq_s = wq[perm], bq[perm]

    # Zero-padded halves: qA keeps lower 16 of each 32-strip, qB the upper.
    # Zeroing weights AND bias makes conv output exactly 0 there; hsw(0)=0.
    half_lo = np.zeros(128, dtype=np.float32)
    for g in range(4):
        half_lo[32 * g: 32 * g + 16] = 1.0
    wqA = wq_s * half_lo[:, None]
    wqB = wq_s * (1.0 - half_lo)[:, None]
    bqA = bq_s * half_lo
    bqB = bq_s * (1.0 - half_lo)

    # lhsT layouts: [K-chunk, 128, M], fp16
    wkT = np.ascontiguousarray(wk_s.T.reshape(2, 128, 128).astype(np.float16))
    wqTA = np.ascontiguousarray(wqA.T.reshape(2, 128, 128).astype(np.float16))
    wqTB = np.ascontiguousarray(wqB.T.reshape(2, 128, 128).astype(np.float16))
    wvT = np.ascontiguousarray(wv.T.reshape(2, 128, 256).astype(np.float16))
    # proj: stacked quads. ao_stack0 rows 32g+d = head g dim d (= input chans
    # 0..127 in original order); stack1 = heads 4..7 (chans 128..255).
    # ao carries C^2/6 * hsw(out) with C=32 (the C recentres 1/(36Z) into
    # fp16-normal range); fold 6/C^2 into the proj weights.
    wpT = np.ascontiguousarray(
        ((6.0 / 1024.0) * wp).T.reshape(2, 128, 512).astype(np.float16))

    # replicated-bias weights (b/128 in every one of the 128 K rows)
    def brep(b):
        return np.ascontiguousarray(
            np.broadcast_to(b[None, :] / 128.0, (128, b.shape[0])
                            ).astype(np.float16))

    # bias transposed, pair-blocked: ebT[p, g, mc, 196*s + n] =
    # (36*bias/SCALE)[g+4s, n, mc*112 + p], rows 112:127 zero (K=128 pad)
    idxs = np.asarray(inputs["bias_idxs"])                 # [196, 784] int32
    ab = np.asarray(inputs["ab"], dtype=np.float32)        # [8, n_off]
    bias = ab[:, idxs]                                     # [8, 196, 784]
    b36 = (36.0 * bias / SCALE)
    # b36[h, n, m]; h = g + 4s; m = mc*112 + p
    b5 = b36.reshape(2, 4, N_TOK_, MC, MCS)                # [s, g, n, mc, p]
    ebT = np.zeros((128, 4, MC, 2 * N_TOK_), dtype=np.float16)
    ebT[:MCS] = b5.transpose(4, 1, 3, 0, 2).reshape(MCS, 4, MC, 2 * N_TOK_)
    ebT = np.ascontiguousarray(ebT.reshape(128, 4, MC * 2 * N_TOK_))

    ident = np.zeros((128, MCS), dtype=np.float16)
    ident[:MCS] = np.eye(MCS, dtype=np.float16)

    xf = x.reshape(B, 256, N_TOK)

    shared = dict(
        wkT=wkT, wqTA=wqTA, wqTB=wqTB, wvT=wvT, wpT=wpT,
        brepk=brep(bk_s), brepqa=brep(bqA), brepqb=brep(bqB),
        brepv=brep(bv), brepp=brep(bp),
        ebT=ebT, ident=ident,
    )
    in_maps = []
    for c in range(N_CORES):
        sl = slice(c * B_PER_CORE, (c + 1) * B_PER_CORE)
        m = dict(shared)
        m["x"] = np.ascontiguousarray(
            xf[sl].reshape(B_PER_CORE, 2, 128, N_TOK).astype(np.float16))
        in_maps.append(m)
    return in_maps


def build_bass(repeat=1):
    nc = bass.Bass()

    x_d = nc.dram_tensor("x", [B_PER_CORE, 2, 128, N_TOK], FP16, kind="ExternalInput")
    wkT_d = nc.dram_tensor("wkT", [2, 128, 128], FP16, kind="ExternalInput")
    wqTA_d = nc.dram_tensor("wqTA", [2, 128, 128], FP16, kind="ExternalInput")
    wqTB_d = nc.dram_tensor("wqTB", [2, 128, 128], FP16, kind="ExternalInput")
    wvT_d = nc.dram_tensor("wvT", [2, 128, 256], FP16, kind="ExternalInput")
    wpT_d = nc.dram_tensor("wpT", [2, 128, 512], FP16, kind="ExternalInput")
    brepk_d = nc.dram_tensor("brepk", [128, 128], FP16, kind="ExternalInput")
    brepqa_d = nc.dram_tensor("brepqa", [128, 128], FP16, kind="ExternalInput")
    brepqb_d = nc.dram_tensor("brepqb", [128, 128], FP16, kind="ExternalInput")
    brepv_d = nc.dram_tensor("brepv", [128, 256], FP16, kind="ExternalInput")
    brepp_d = nc.dram_tensor("brepp", [128, 512], FP16, kind="ExternalInput")
    ebT_d = nc.dram_tensor("ebT", [128, 4, MC * 2 * N_TOK_], FP16, kind="ExternalInput")
    ident_d = nc.dram_tensor("ident", [128, MCS], FP16, kind="ExternalInput")
    n_img = repeat * B_PER_CORE
    y_d = nc.dram_tensor("y", [n_img, 128, 4, N_TOK_], F32, kind="ExternalOutput")

    with tile.TileContext(nc) as tc:
        with (
            tc.tile_pool(name="consts", bufs=1) as consts,
            tc.tile_pool(name="xp", bufs=8) as xp,
            tc.tile_pool(name="kp", bufs=3) as kp,
            tc.tile_pool(name="qmp", bufs=2) as qmp,
            tc.tile_pool(name="vp", bufs=3) as vp,
            tc.tile_pool(name="tp", bufs=6) as tp,
            tc.tile_pool(name="ep", bufs=16) as ep,
            tc.tile_pool(name="att", bufs=2) as attp,
            tc.tile_pool(name="yp", bufs=8) as yp,
            tc.tile_pool(name="psc", bufs=2, space="PSUM") as psc,
            tc.tile_pool(name="pss", bufs=4, space="PSUM") as pss,
            tc.tile_pool(name="psa", bufs=1, space="PSUM") as psa,
            tc.tile_pool(name="psz", bufs=1, space="PSUM") as psz,
        ):
            # ---- load weights / constants (resident) ----
            wk = [consts.tile([128, 128], FP16, tag=f"wk{i}", name=f"wk{i}") for i in range(2)]
            wqa = [consts.tile([128, 128], FP16, tag=f"wqa{i}", name=f"wqa{i}") for i in range(2)]
            wqb = [consts.tile([128, 128], FP16, tag=f"wqb{i}", name=f"wqb{i}") for i in range(2)]
            wv = [consts.tile([128, 256], FP16, tag=f"wv{i}", name=f"wv{i}") for i in range(2)]
            wp = [consts.tile([128, 512], FP16, tag=f"wp{i}", name=f"wp{i}") for i in range(2)]
            for i in range(2):
                nc.sync.dma_start(out=wk[i], in_=wkT_d[i])
                nc.sync.dma_start(out=wqa[i], in_=wqTA_d[i])
                nc.sync.dma_start(out=wqb[i], in_=wqTB_d[i])
                nc.sync.dma_start(out=wv[i], in_=wvT_d[i])
                nc.sync.dma_start(out=wp[i], in_=wpT_d[i])
            brepk = consts.tile([128, 128], FP16, tag="brepk", name="brepk")
            brepqa = consts.tile([128, 128], FP16, tag="brepqa", name="brepqa")
            brepqb = consts.tile([128, 128], FP16, tag="brepqb", name="brepqb")
            brepv = consts.tile([128, 256], FP16, tag="brepv", name="brepv")
            brepp = consts.tile([128, 512], FP16, tag="brepp", name="brepp")
            nc.sync.dma_start(out=brepk, in_=brepk_d[:])
            nc.sync.dma_start(out=brepqa, in_=brepqa_d[:])
            nc.sync.dma_start(out=brepqb, in_=brepqb_d[:])
            nc.sync.dma_start(out=brepv, in_=brepv_d[:])
            nc.sync.dma_start(out=brepp, in_=brepp_d[:])
            ebT = consts.tile([128, 4, MC * 2 * N_TOK_], FP16, tag="ebT", name="ebT")
            for g in range(4):
                nc.sync.dma_start(out=ebT[:, g: g + 1, :],
                                  in_=ebT_d[:, g: g + 1, :])
            ident = consts.tile([128, MCS], FP16, tag="ident", name="ident")
            nc.sync.dma_start(out=ident, in_=ident_d[:])

            ones_f = consts.tile([128, 392], FP16, tag="ones_f", name="ones_f")
            nc.vector.memset(ones_f, 1.0)
            # per-pair ACT-side absorber outputs (never reused)
            scr_act = consts.tile([1, n_img * 20], F32, tag="scr_act",
                                  name="scr_act")

            # Wait-absorbers: walrus allows at most ONE sync wait per
            # instruction. A standalone ldweights reading one element of a
            # tile ticks the PE clock after that tile's producer; later PE
            # instructions then inherit the dep through the engine clock.
            # (fp16 tiles only -- ldweights refuses fp32.)
            def absorb(t, tile_position=None):
                sl = t[tuple(slice(0, 1) for _ in t.shape)]
                return nc.tensor.ldweights(weights=sl, tile_position=tile_position)

            for t in (wk[0], wk[1], wqa[0], wqa[1], wqb[0], wqb[1],
                      wv[0], wv[1], wp[0], wp[1],
                      brepk, brepqa, brepqb, brepv, brepp, ident, ones_f):
                absorb(t)
            absorb(ebT)

            # ---- one-time slot zero-inits ----
            # qm slots: 4 tags x 2 bufs, fully zeroed; per-image writes only
            # touch the strip rows, so the masked regions stay zero.
            for i in range(2):
                for g in range(4):
                    t = qmp.tile([128, 2 * N_TOK_], FP16, tag=f"qm{g}",
                                 name=f"qm{g}")
                    nc.vector.memset(t, 0.0)
            # E slots: rows 96:128 zeroed (exp rewrites 96:112 every use;
            # 112:127 stay zero = K-pad for the combine matmuls).
            for i in range(16):
                t = ep.tile([128, 2 * N_TOK_], FP16, tag="E", name="E")
                nc.vector.memset(t[96:128, :], 0.0)
            # zero bias column for ACT ops: avoids the auto const-AP, whose
            # writer sits on a separate ACT queue and costs an extra wait.
            # Memset LAST among the DVE inits; the ACT copy below observes its
            # tick so the first exps' WAW against the slot memsets elides.
            bias0 = consts.tile([128, 1], F32, tag="bias0", name="bias0")
            nc.vector.memset(bias0, 0.0)
            nc.scalar.copy(scr_act[0:1, n_img * 8: n_img * 8 + 1],
                           bias0[0:1, 0:1])

            # ---------- per-image emission helpers ----------
            xt_cache = {}

            def make_conv_units(b, b_seq=0):
                """DMA x and build conv-chunk closures for image b. Returns
                (tiles dict, unit list); units are emitted interleaved into
                the previous image's attention phase."""
                if b_seq < 16:
                    xt = [xp.tile([128, N_TOK], FP16, tag=f"x{i}", name=f"x{i}",
                                  bufs=min(8 * repeat, 16))
                          for i in range(2)]
                    xt_cache[b_seq % 16] = xt
                    for i in range(2):
                        nc.sync.dma_start(out=xt[i], in_=x_d[b, i])
                        absorb(xt[i])
                else:
                    xt = xt_cache[b_seq % 16]
                xst = [bass.AP(tensor=xt[i].tensor, offset=xt[i].offset,
                               ap=[xt[i].ap[0], [2 * RES, RES_], [2, RES_]])
                       for i in range(2)]
                k_sb = kp.tile([128, N_TOK], FP16, tag="k", name="k")
                qm = [qmp.tile([128, 2 * N_TOK_], FP16, tag=f"qm{g}",
                               name=f"qm{g}") for g in range(4)]
                vt = vp.tile([128, MC, 4, 68], FP16, tag="vt", name="vt")

                units = []

                def vt_init():
                    def emit():
                        # K-pad rows zero (also kills fp16 garbage that would
                        # NaN the 0*x products), then the two 36.0 Z columns.
                        nc.vector.memset(vt[96:128, :, :, :], 0.0)
                        nc.vector.memset(vt[:, :, :, 32:33], 36.0)
                        nc.vector.memset(vt[:, :, :, 66:67], 36.0)
                    return emit

                def k_half(nh):
                    def emit():
                        pk = psc.tile([128, 392], F32, tag="conv", name="conv")
                        sl = slice(nh * 392, (nh + 1) * 392)
                        nc.tensor.matmul(pk, wk[0], xt[0][:, sl], start=True, stop=False)
                        nc.tensor.matmul(pk, wk[1], xt[1][:, sl], start=False, stop=False)
                        nc.tensor.matmul(pk, brepk, ones_f[:, :392], start=False, stop=True)
                        tk = tp.tile([128, 392], FP16, tag="tk", name="tk")
                        nc.vector.tensor_scalar(tk, pk, -3.0, 3.0, ALU.max, ALU.min)
                        nc.vector.scalar_tensor_tensor(
                            k_sb[:, sl], tk, 3.0, pk, ALU.add, ALU.mult)
                    return emit

                def q_conv(wset, bset, col0):
                    def emit():
                        pq = psc.tile([128, 392], F32, tag="conv", name="conv")
                        pq196 = pq[:, :N_TOK_]
                        nc.tensor.matmul(pq196, wset[0], xst[0], start=True, stop=False)
                        nc.tensor.matmul(pq196, wset[1], xst[1], start=False, stop=False)
                        nc.tensor.matmul(pq196, bset, ones_f[:, :N_TOK_],
                                         start=False, stop=True)
                        tq = tp.tile([128, 392], FP16, tag="tk", name="tq")
                        tq196 = tq[:, :N_TOK_]
                        nc.vector.tensor_scalar(tq196, pq196, -3.0, 3.0,
                                                ALU.max, ALU.min)
                        # write each 32-strip into its pair's masked tile;
                        # the conv's zeroed weights make the unused half of
                        # each strip exactly 0.
                        for g in range(4):
                            rows = slice(32 * g, 32 * g + 32)
                            nc.vector.scalar_tensor_tensor(
                                qm[g][rows, col0: col0 + N_TOK_],
                                tq196[rows, :], 3.0, pq196[rows, :],
                                ALU.add, ALU.mult)
                    return emit

                def v_chunk(mc):
                    def emit():
                        pv = psc.tile([128, 392], F32, tag="conv", name="conv")
                        pv = pv[:MCS, :256]
                        msl = slice(mc * MCS, (mc + 1) * MCS)
                        nc.tensor.matmul(pv, xt[0][:, msl], wv[0], start=True, stop=False)
                        nc.tensor.matmul(pv, xt[1][:, msl], wv[1], start=False, stop=False)
                        nc.tensor.matmul(pv, ones_f[:, :MCS], brepv, start=False, stop=True)
                        tv = tp.tile([MCS, 256], FP16, tag="tv", name="tv")
                        nc.vector.tensor_scalar(tv, pv, -3.0, 3.0, ALU.max, ALU.min)
                        nc.vector.scalar_tensor_tensor(
                            vt[:MCS, mc, :, 0:32],
                            tv[:, 0:128].rearrange("p (g d) -> p g d", g=4),
                            3.0,
                            pv[:, 0:128].rearrange("p (g d) -> p g d", g=4),
                            ALU.add, ALU.mult)
                        nc.vector.scalar_tensor_tensor(
                            vt[:MCS, mc, :, 34:66],
                            tv[:, 128:256].rearrange("p (g d) -> p g d", g=4),
                            3.0,
                            pv[:, 128:256].rearrange("p (g d) -> p g d", g=4),
                            ALU.add, ALU.mult)
                    return emit

                units.append(vt_init())
                units.append(k_half(0))
                units.append(k_half(1))
                units.append(q_conv(wqa, brepqa, 0))
                units.append(q_conv(wqb, brepqb, N_TOK_))
                for mc in range(MC):
                    units.append(v_chunk(mc))
                return dict(k=k_sb, qm=qm, vt=vt), units

            state = dict(prev_u6=None, prev_oc=None)

            def emit_attention(b, tiles, units):
                """Attention + proj for image b; `units` (next image's conv
                chunks) are interleaved into the stalls."""
                k_sb, qm, vt = tiles["k"], tiles["qm"], tiles["vt"]
                ui = iter(units)

                def emit_next_unit(n=1):
                    for _ in range(n):
                        u = next(ui, None)
                        if u is not None:
                            u()

                # PE observes the conv-epilogue DVE ticks once (vt is written
                # last among conv outputs; k/qm ticks are older)
                absorb(vt)
                o_sbs = []
                zall = attp.tile([1, 8 * N_TOK_], F32, tag="zall", name="zall")
                for g in range(4):
                    Es = []
                    # two waves (4+3 chunks): all of a wave's score matmuls,
                    # then ONE ACT absorber takes the PE tick, then the exps
                    # carry only their (non-elidable same-proc) E-slot WAW.
                    for wv, mcs in ((0, range(0, 4)), (1, range(4, MC))):
                        scs = []
                        for mc in mcs:
                            sc = pss.tile([MCS, 2 * N_TOK_], F32, tag="score",
                                          name="score")
                            esl = slice(mc * 2 * N_TOK_, (mc + 1) * 2 * N_TOK_)
                            nc.tensor.matmul(sc, ident, ebT[:, g, esl],
                                             start=True, stop=False,
                                             skip_group_check=True)
                            msl = slice(mc * MCS, (mc + 1) * MCS)
                            nc.tensor.matmul(sc, k_sb[:, msl], qm[g],
                                             start=False, stop=True,
                                             skip_group_check=True)
                            scs.append(sc)
                        aidx = n_img * 8 + 8 + b * 8 + g * 2 + wv
                        wgate = nc.scalar.copy(scr_act[0:1, aidx: aidx + 1],
                                               scs[-1][0:1, 0:1])
                        for i, mc in enumerate(mcs):
                            E = ep.tile([128, 2 * N_TOK_], FP16, tag="E",
                                        name="E")
                            ei = nc.scalar.activation(E[:MCS, :], scs[i],
                                                      ACTF.Exp,
                                                      bias=bias0[:MCS, :],
                                                      scale=SCALE / 36.0)
                            add_dep_helper(ei.ins, wgate.ins, True,
                                           "exps after wave gate")
                            Es.append(E)
                    emit_next_unit()
                    po = psa.tile([97, N_TOK_], F32, tag="att", name="att")
                    # PE dummy takes the po slot WAW; ogate (ldweights) takes
                    # the o_sb-copy ACT tick first so the dummy carries only
                    # the PE wait. Real combine matmuls then wait only on E.
                    ogate = None
                    if state["prev_oc"] is not None:
                        ogate = absorb(ones_f)
                        add_dep_helper(ogate.ins, state["prev_oc"].ins, True,
                                       "osb copy gate")
                    podum = nc.tensor.matmul(po[0:1, 0:1], ones_f[0:1, 0:1],
                                             ones_f[0:1, 0:1], start=True,
                                             stop=True, skip_group_check=True)
                    if ogate is not None:
                        add_dep_helper(podum.ins, ogate.ins, True,
                                       "osb gate before po dummy")
                    for mc in range(MC):
                        nc.tensor.matmul(po[64:97, :], vt[:, mc, g, 34:67],
                                         Es[mc][:, N_TOK_:],
                                         start=(mc == 0), stop=False,
                                         skip_group_check=True)
                        nc.tensor.matmul(po[0:33, :], vt[:, mc, g, 0:33],
                                         Es[mc][:, :N_TOK_],
                                         start=(mc == 0), stop=(mc == MC - 1),
                                         skip_group_check=True)
                    # ACT observes the latest DVE epilogue tick so the o_sb
                    # copy's slot WAR elides to a single PE wait
                    if state["prev_u6"] is not None:
                        nc.scalar.copy(scr_act[0:1, b * 8 + g: b * 8 + g + 1],
                                       state["prev_u6"][0:1, 0:1])
                        state["prev_u6"] = None
                    o_sb = attp.tile([97, N_TOK_], F32, tag="o", name="o", bufs=8)
                    oc = nc.scalar.copy(o_sb, po)
                    state["prev_oc"] = oc
                    o_sbs.append(o_sb)
                    # gather the two Z rows into zall (cross-partition-base
                    # DVE copies)
                    zsl = slice((2 * g) * N_TOK_, (2 * g + 1) * N_TOK_)
                    nc.vector.tensor_copy(out=zall[0:1, zsl], in_=o_sb[32:33, :])
                    zsl = slice((2 * g + 1) * N_TOK_, (2 * g + 2) * N_TOK_)
                    nc.vector.tensor_copy(out=zall[0:1, zsl], in_=o_sb[96:97, :])
                    emit_next_unit()
                # C/(36Z) for all 8 heads (C=32 keeps rz16 fp16-normal):
                # ln(36Z/32) then exp(-x), fp16 out
                lnz = attp.tile([1, 8 * N_TOK_], F32, tag="lnz", name="lnz")
                nc.scalar.activation(lnz, zall, ACTF.Ln, bias=bias0[0:1, :],
                                     scale=1.0 / 32.0)
                rz16 = attp.tile([1, 8 * N_TOK_], FP16, tag="rz16", name="rz16")
                nc.scalar.activation(rz16, lnz, ACTF.Exp, bias=bias0[0:1, :],
                                     scale=-1.0)

                ao_stack = [
                    attp.tile([128, N_TOK_], FP16, tag="ao0", name="ao0"),
                    attp.tile([128, N_TOK_], FP16, tag="ao1", name="ao1"),
                ]
                for g in range(4):
                    if state["prev_u6"] is not None:
                        # PE observes the last u6 tick so the pzb slot-reuse
                        # WAR elides
                        absorb(state["prev_u6"])
                    pzb = psz.tile([32, 2 * N_TOK_], F32, tag="zb", name="zb")
                    nc.tensor.matmul(pzb, ones_f[0:1, 0:32],
                                     rz16[0:1, 2 * g * N_TOK_:
                                          (2 * g + 2) * N_TOK_],
                                     start=True, stop=True,
                                     skip_group_check=True)
                    o_sb = o_sbs[g]
                    for s in range(2):
                        rows = slice(64 * s, 64 * s + 32)
                        nsl = slice(s * N_TOK_, (s + 1) * N_TOK_)
                        u6 = attp.tile([32, N_TOK_], FP16, tag="u6", name="u6",
                                       bufs=8)
                        nc.vector.tensor_mul(u6, o_sb[rows, :], pzb[:, nsl])
                        c1 = attp.tile([32, N_TOK_], FP16, tag="c1", name="c1",
                                       bufs=8)
                        # u6 = C*out/6; ao = min(c1, C)*u6 = C^2*hsw(out)/6
                        nc.vector.tensor_scalar(c1, u6, 16.0, 0.0, ALU.add, ALU.max)
                        nc.vector.scalar_tensor_tensor(
                            ao_stack[s][32 * g: 32 * g + 32, :], c1, 32.0, u6,
                            ALU.min, ALU.mult)
                        state["prev_u6"] = u6
                    emit_next_unit()

                # ---- proj conv: 2 stacked K=128 matmuls per 128-out chunk ----
                y_sb = yp.tile([128, 4, N_TOK_], F32,
                               tag="y" if b < B_PER_CORE else "y2", name="y",
                               bufs=8)
                for mt in range(4):
                    pp = psc.tile([128, 392], F32, tag="conv", name="conv")
                    pp196 = pp[:, :N_TOK_]
                    csl = slice(mt * 128, (mt + 1) * 128)
                    for st in range(2):
                        nc.tensor.matmul(pp196, wp[st][:, csl], ao_stack[st],
                                         start=(st == 0), stop=False,
                                         skip_group_check=True)
                    nc.tensor.matmul(pp196, brepp[:, csl], ones_f[:, :N_TOK_],
                                     start=False, stop=True,
                                     skip_group_check=True)
                    tpj = tp.tile([128, 392], FP16, tag="tk", name="tpj")
                    tpj196 = tpj[:, :N_TOK_]
                    nc.vector.tensor_scalar(tpj196, pp196, -3.0, 3.0,
                                            ALU.max, ALU.min)
                    nc.vector.scalar_tensor_tensor(
                        y_sb[:, mt, :], tpj196, 3.0, pp196, ALU.add, ALU.mult)
                    emit_next_unit()
                if b < B_PER_CORE:
                    nc.gpsimd.dma_start(out=y_d[b], in_=y_sb)
                # drain any units the loop didn't consume
                emit_next_unit(len(units))

            # ---------- main pipeline ----------
            tiles, units = make_conv_units(0, 0)
            for u in units:
                u()
            for bb in range(n_img):
                if bb + 1 < n_img:
                    ntiles, nunits = make_conv_units((bb + 1) % B_PER_CORE, bb + 1)
                else:
                    ntiles, nunits = None, []
                emit_attention(bb, tiles, nunits)
                tiles = ntiles

    return nc


def _postprocess(y):
    # y dram: [8, 128, 4, 196] holding 6*hsw -> [8, 512, 14, 14]
    return y.transpose(0, 2, 1, 3).reshape(B_PER_CORE, 512, RES_, RES_) * (1.0 / 6.0)


def _run(inputs, trace=False):
    in_maps = _prepare_host(inputs)
    if "nc" not in _CACHE:
        _CACHE["nc"] = build_bass()
    nc = _CACHE["nc"]
    res = run_bass_kernel_spmd(nc, in_maps, list(range(N_CORES)), trace=trace)
    outs = []
    for c in range(N_CORES):
        outs.append(_postprocess(np.asarray(res.results[c]["y"])))
    full = np.concatenate(outs, axis=0).astype(np.float32)
    return full, res


def kernel(**inputs) -> np.ndarray:
    full, _ = _run(inputs, trace=False)
    return full
